# revision 9
# baseline (speedup 1.0000x reference)
"""Trainium2 Bass kernel for AgentCapabilityEstimator (dense MLP, 3 heads).

Reference computation (B=16384, OBS=512, H=1024, N=9):
    g  = relu(relu(obs @ W1 + b1) @ W2 + b2)                    [B, H]
    cov  = sigmoid(relu(g @ Wc1 + bc1) @ Wc2 + bc2)             [B, 1]
    trk  = sigmoid(relu(g @ Wt1 + bt1) @ Wt2 + bt2)             [B, 1]
    coop = sigmoid(relu([g,g] @ Wk1 + bk1) @ Wk2 + bk2)         [B, 1]
    outputs broadcast to [B, 9] each.

Strategy: pure data parallelism over 8 cores (2048 rows each). Activations
kept feature-major ([feature_chunk=128 partitions, batch free dim]) so every
layer is a chain of 128x128 stationary x [128, 512] moving f32r matmuls with
natural-layout weights. Host-side prep folds Wk1 ([g,g] @ Wk1 == g @ (Wk1_hi
+ Wk1_lo)), concatenates the three head hidden layers into one [1024, 2048]
GEMM, and packs the three scalar head outputs into one [2048, 3] block-sparse
final matmul.
"""

import numpy as np

import concourse.bass as bass
import concourse.mybir as mybir
import concourse.tile as tile
from concourse import bacc
from concourse.bass_utils import run_bass_kernel_spmd
from concourse.masks import make_identity

B, OBS, H, N = 16384, 512, 1024, 9
NCORES = 8
BC = B // NCORES          # 2048 batch rows per core
NTILE = 512               # batch rows per compute tile
NT = BC // NTILE          # 4 tiles per core
P = 128
KO = OBS // P             # 4 obs chunks
HO = H // P               # 8 hidden chunks
AO = 2 * H // P           # 16 chunks of the stacked head-hidden features
F32 = mybir.dt.float32
F32R = mybir.dt.float32r

RELU = mybir.ActivationFunctionType.Relu
SIGMOID = mybir.ActivationFunctionType.Sigmoid


def build_nc() -> bass.Bass:
    nc = bacc.Bacc(trn_type="TRN2", target_bir_lowering=False, debug=False)

    obsT = nc.dram_tensor("obsT", [OBS, BC], F32R, kind="ExternalInput").ap()
    W1 = nc.dram_tensor("W1", [OBS, H], F32R, kind="ExternalInput").ap()
    W2 = nc.dram_tensor("W2", [H, H], F32R, kind="ExternalInput").ap()
    Wh = nc.dram_tensor("Wh", [H, 2 * H], F32R, kind="ExternalInput").ap()
    Wfin = nc.dram_tensor("Wfin", [2 * H, 3], F32R, kind="ExternalInput").ap()
    b1 = nc.dram_tensor("b1", [H], F32, kind="ExternalInput").ap()
    b2 = nc.dram_tensor("b2", [H], F32, kind="ExternalInput").ap()
    bh = nc.dram_tensor("bh", [2 * H], F32, kind="ExternalInput").ap()
    bfin = nc.dram_tensor("bfin", [3], F32, kind="ExternalInput").ap()
    out_cov = nc.dram_tensor("cov", [BC, N], F32, kind="ExternalOutput").ap()
    out_trk = nc.dram_tensor("trk", [BC, N], F32, kind="ExternalOutput").ap()
    out_coop = nc.dram_tensor("coop", [BC, N], F32, kind="ExternalOutput").ap()

    with tile.TileContext(nc) as tc:
        _body(tc, obsT, W1, W2, Wh, Wfin, b1, b2, bh, bfin,
              out_cov, out_trk, out_coop)
    nc.compile()
    return nc


def _body(tc, obsT, W1, W2, Wh, Wfin, b1, b2, bh, bfin,
          out_cov, out_trk, out_coop):
    nc = tc.nc

    with (
        tc.tile_pool(name="weights", bufs=1) as wpool,
        tc.tile_pool(name="obs", bufs=1) as obspool,
        tc.tile_pool(name="acts", bufs=1) as actpool,
        tc.tile_pool(name="small", bufs=4) as smallpool,
        tc.tile_pool(name="psum", bufs=6, space="PSUM") as psum,
        tc.tile_pool(name="psum_small", bufs=1, space="PSUM") as psum_s,
    ):
        # ---- resident weights / biases ----------------------------------
        w1_sb = wpool.tile([P, KO, H], F32R)
        nc.sync.dma_start(out=w1_sb, in_=W1.rearrange("(c p) h -> p c h", p=P))
        w2_sb = wpool.tile([P, HO, H], F32R)
        nc.sync.dma_start(out=w2_sb, in_=W2.rearrange("(c p) h -> p c h", p=P))
        wh_sb = wpool.tile([P, HO, 2 * H], F32R)
        nc.sync.dma_start(out=wh_sb, in_=Wh.rearrange("(c p) h -> p c h", p=P))
        wfin_sb = wpool.tile([P, AO, 3], F32R)
        nc.sync.dma_start(out=wfin_sb, in_=Wfin.rearrange("(c p) m -> p c m", p=P))
        b1_sb = wpool.tile([P, HO], F32)
        nc.sync.dma_start(out=b1_sb, in_=b1.rearrange("(c p) -> p c", p=P))
        b2_sb = wpool.tile([P, HO], F32)
        nc.sync.dma_start(out=b2_sb, in_=b2.rearrange("(c p) -> p c", p=P))
        bh_sb = wpool.tile([P, AO], F32)
        nc.sync.dma_start(out=bh_sb, in_=bh.rearrange("(c p) -> p c", p=P))
        bfin_sb = wpool.tile([3, 1], F32)
        nc.sync.dma_start(out=bfin_sb, in_=bfin.rearrange("(m o) -> m o", o=1))
        ident = wpool.tile([P, P], F32)
        make_identity(nc, ident)

        obsT_r = obsT.rearrange("(c p) b -> p c b", p=P)

        for t in range(NT):
            bs = t * NTILE
            # ---- load obs tile (feature-major, pre-transposed on host) --
            x = obspool.tile([P, KO, NTILE], F32R)
            nc.sync.dma_start(out=x, in_=obsT_r[:, :, bs:bs + NTILE])

            # ---- layer 1: g1 = relu(W1.T @ x + b1) ----------------------
            g1 = actpool.tile([P, HO, NTILE], F32R, tag="g1")
            for m in range(HO):
                ps = psum.tile([P, NTILE], F32, tag="mm")
                for k in range(KO):
                    nc.tensor.matmul(
                        ps, w1_sb[:, k, m * P:(m + 1) * P], x[:, k, :],
                        start=(k == 0), stop=(k == KO - 1))
                nc.scalar.activation(g1[:, m, :], ps, RELU,
                                     bias=b1_sb[:, m:m + 1])

            # ---- layer 2: g = relu(W2.T @ g1 + b2) ----------------------
            g = actpool.tile([P, HO, NTILE], F32R, tag="g")
            for m in range(HO):
                ps = psum.tile([P, NTILE], F32, tag="mm")
                for k in range(HO):
                    nc.tensor.matmul(
                        ps, w2_sb[:, k, m * P:(m + 1) * P], g1[:, k, :],
                        start=(k == 0), stop=(k == HO - 1))
                nc.scalar.activation(g[:, m, :], ps, RELU,
                                     bias=b2_sb[:, m:m + 1])

            # ---- head hiddens: h = relu(Wh.T @ g + bh) ------------------
            h = actpool.tile([P, AO, NTILE], F32R, tag="h")
            for m in range(AO):
                ps = psum.tile([P, NTILE], F32, tag="mm")
                for k in range(HO):
                    nc.tensor.matmul(
                        ps, wh_sb[:, k, m * P:(m + 1) * P], g[:, k, :],
                        start=(k == 0), stop=(k == HO - 1))
                nc.scalar.activation(h[:, m, :], ps, RELU,
                                     bias=bh_sb[:, m:m + 1])

            # ---- head finals: sig = sigmoid(Wfin.T @ h + bfin) [3,NTILE]
            ps3 = psum_s.tile([3, NTILE], F32, tag="fin")
            for k in range(AO):
                nc.tensor.matmul(ps3, wfin_sb[:, k, :], h[:, k, :],
                                 start=(k == 0), stop=(k == AO - 1))
            sig = smallpool.tile([3, NTILE], F32, tag="sig")
            nc.scalar.activation(sig, ps3, SIGMOID, bias=bfin_sb[0:3, 0:1])

            # ---- transpose to batch-major, broadcast to 9, store --------
            for c in range(NTILE // P):
                pst = psum_s.tile([P, 3], F32, tag="tr")
                nc.tensor.transpose(pst, sig[:, c * P:(c + 1) * P],
                                    ident[0:3, 0:3])
                o27 = smallpool.tile([P, 3, N], F32, tag="o27")
                for i in range(3):
                    nc.vector.tensor_copy(
                        out=o27[:, i, :],
                        in_=pst[:, i:i + 1].broadcast_to([P, N]))
                rows = slice(bs + c * P, bs + (c + 1) * P)
                nc.sync.dma_start(out=out_cov[rows, :], in_=o27[:, 0, :])
                nc.sync.dma_start(out=out_trk[rows, :], in_=o27[:, 1, :])
                nc.sync.dma_start(out=out_coop[rows, :], in_=o27[:, 2, :])


_NC_CACHE = None


def _get_nc() -> bass.Bass:
    global _NC_CACHE
    if _NC_CACHE is None:
        _NC_CACHE = build_nc()
    return _NC_CACHE


def prep_inputs(obs, W1, b1, W2, b2, Wc1, bc1, Wc2, bc2,
                Wt1, bt1, Wt2, bt2, Wk1, bk1, Wk2, bk2, **_unused):
    """Host-side prep: fold/concat weights, transpose obs, build shards."""
    f = np.float32
    obsT = np.ascontiguousarray(np.asarray(obs, f).T)          # [OBS, B]
    Wk1f = np.asarray(Wk1[:H], f) + np.asarray(Wk1[H:], f)     # [H, H]
    Wh = np.ascontiguousarray(
        np.concatenate([np.asarray(Wc1, f), np.asarray(Wt1, f), Wk1f],
                       axis=1))                                # [H, 2H]
    Wfin = np.zeros((2 * H, 3), f)
    Wfin[0:H // 2, 0] = np.asarray(Wc2, f)[:, 0]
    Wfin[H // 2:H, 1] = np.asarray(Wt2, f)[:, 0]
    Wfin[H:2 * H, 2] = np.asarray(Wk2, f)[:, 0]
    bh = np.concatenate([np.asarray(bc1, f), np.asarray(bt1, f),
                         np.asarray(bk1, f)])                  # [2H]
    bfin = np.array([np.asarray(bc2, f)[0], np.asarray(bt2, f)[0],
                     np.asarray(bk2, f)[0]], f)

    shared = dict(
        W1=np.ascontiguousarray(np.asarray(W1, f)),
        W2=np.ascontiguousarray(np.asarray(W2, f)),
        Wh=Wh, Wfin=Wfin,
        b1=np.ascontiguousarray(np.asarray(b1, f)),
        b2=np.ascontiguousarray(np.asarray(b2, f)),
        bh=np.ascontiguousarray(bh), bfin=bfin,
    )
    in_maps = []
    for c in range(NCORES):
        m = dict(shared)
        m["obsT"] = np.ascontiguousarray(obsT[:, c * BC:(c + 1) * BC])
        in_maps.append(m)
    return in_maps


def kernel(**inputs):
    nc = _get_nc()
    in_maps = prep_inputs(**inputs)
    res = run_bass_kernel_spmd(nc, in_maps, list(range(NCORES))).results
    cov = np.concatenate([res[c]["cov"] for c in range(NCORES)], axis=0)
    trk = np.concatenate([res[c]["trk"] for c in range(NCORES)], axis=0)
    coop = np.concatenate([res[c]["coop"] for c in range(NCORES)], axis=0)
    return (cov, trk, coop)
